# revision 12
# baseline (speedup 1.0000x reference)
"""Trainium2 Bass kernel for per-position channel-mixing layer.

Reference computation (B=128, C=32, H=W=64, L=H*W=4096):
    out[b, :, l] = W[l].T @ x[b, :, l] + bias[l]      W[l]: [C, C] per position

Strategy:
  - Shard the spatial L dim across 8 cores (512 positions each); all device
    tensors in bf16 (fp32 PSUM accumulation) to halve HBM traffic; the
    harness gate is rel_err < 2e-2 and bf16 lands ~3.5e-3.
  - x-stationary matmuls at a single PE tile position (0,0): lhsT = x[l]
    [32c, 128b] (stationary, LS pipelined ~27ns/matmul), rhs = W[l]
    [32, 32] (moving) -> psum[b, c'] streams only 32 PE columns per
    position (4x less than W-stationary).  HW constraint discovered by
    bisection: consecutive matmuls whose PE tiles overlap in columns but
    sit at different row offsets hang the device, so everything stays at
    (0,0) with x and W on SBUF partitions 0-31.
  - 16 positions per PSUM bank [128, 512]; bias is accumulated into each
    bank by one K=32 matmul (stationary = const 1/32 [32,128], moving =
    bias replicated on 32 partitions -> sum_c (1/32)*bias = bias exactly).
    The 32-partition bias replica is built once by a log-tree of
    SBUF->SBUF DMAs during the ramp.  Bank 0 instead uses a K=1 ones
    matmul straight off the HBM-loaded partition-0 row, so chunk 0 never
    waits for the tree.
  - Eviction is then a pure fp32->bf16 copy [128, 512], alternating
    Vector (tensor_copy) and Scalar (activation Copy).
  - DMA queue budget: x+w loads on sync, stores + bias tree on gpsimd,
    bias load on scalar; each dma_start costs ~700ns of issuing-engine
    time, so they are spread.  Stores are split into <=4-bank (512KB)
    segments; late-chunk stores alternate across rings.
"""

import numpy as np

B, C, H, W = 128, 32, 64, 64
L = H * W                 # 4096
N_CORES = 8
L_CORE = L // N_CORES     # 512 positions per core
BANK_POS = 16             # positions per PSUM bank ([128, 512] fp32)
# positions per DMA chunk (sum = 512); small edges shorten ramp-up/down
CHUNK_POS = [16, 32, 64, 96, 96, 96, 80, 32]
assert sum(CHUNK_POS) == L_CORE and all(p % BANK_POS == 0 for p in CHUNK_POS)
CHUNK_BANKS = [p // BANK_POS for p in CHUNK_POS]
X_LEN = L_CORE * C * B                          # flat elem count per core
W_LEN = L_CORE * C * C
BIA_LEN = L_CORE * C
SEG_BANKS = 4   # store in <=4-bank (512KB bf16) segments

_CACHE = {}


def _split_multi_waits(nc):
    """This container's pinned walrus build rejects instructions carrying
    more than one semaphore wait ("Too many sync wait commands",
    CoreV3GenImpl.cpp:104), while Tile's wait-assignment pass freely
    attaches several. Legalize: hoist all but the last wait of every
    instruction onto single-wait NOPs placed just before it on the same
    engine (sequential waits on one queue are semantically identical)."""
    import concourse.mybir as mybir

    for f in nc.m.functions:
        for bb in f.blocks:
            insts = list(bb.instructions)
            new = []
            changed = False
            for ins in insts:
                si = getattr(ins, "sync_info", None)
                if si is not None and si.on_wait and len(si.on_wait) > 1:
                    waits = list(si.on_wait)
                    for idx, w in enumerate(waits[:-1]):
                        nop = mybir.InstNoOp(
                            name=f"{ins.name}-ws{idx}",
                            ins=[],
                            outs=[],
                            sync_info=mybir.SyncInfo(on_wait=[w], on_update=[]),
                        )
                        nop.engine = ins.engine
                        nc.register_instruction(nop)
                        new.append(nop)
                    si.on_wait = [waits[-1]]
                    changed = True
                new.append(ins)
            if changed:
                bb.instructions = new


def _patch_walrus_flags():
    """Append --enable-remote-semaphore-dma to walrus compiles: replaces the
    finishing CoreBarrier with a DMA semaphore update, trimming ~1.5us off the
    NRT completion sequence. Safe for re-execution: the bass preamble clears
    the kernel sem range at start of every run."""
    import concourse.bass_utils as bu

    if getattr(bu.run_command, "_remote_sem_patch", False):
        return
    _orig = bu.run_command

    def patched(argv, **kw):
        if argv and "walrus_driver" in str(argv[0]):
            argv = list(argv) + ["--enable-remote-semaphore-dma"]
        return _orig(argv, **kw)

    patched._remote_sem_patch = True
    bu.run_command = patched


def _build_nc():
    _patch_walrus_flags()
    import concourse.bass as bass  # noqa: F401  (environment module)
    import concourse.mybir as mybir
    import concourse.tile as tile

    f32 = mybir.dt.float32
    bf16 = mybir.dt.bfloat16
    nc = bass.Bass()
    xin = nc.declare_dram_parameter("xin", [X_LEN], bf16, isOutput=False)
    win = nc.declare_dram_parameter("win", [W_LEN], bf16, isOutput=False)
    bin_ = nc.declare_dram_parameter("bin", [BIA_LEN], bf16, isOutput=False)
    oout = nc.declare_dram_parameter("oout", [X_LEN], bf16, isOutput=True)

    max_p = max(CHUNK_POS)
    with tile.TileContext(nc) as tc:
        with (
            tc.tile_pool(name="xp", bufs=3) as xp,
            tc.tile_pool(name="wp", bufs=3) as wp,
            tc.tile_pool(name="op", bufs=4) as op,
            tc.tile_pool(name="bp", bufs=1) as bp,
            tc.tile_pool(name="cp", bufs=1) as cp,
            tc.tile_pool(name="ps", bufs=8, space="PSUM") as ps,
        ):
            # bias replicated on partitions 0-31 via log-tree of SBUF DMAs
            bt = bp.tile([32, BIA_LEN], bf16)
            nc.scalar.dma_start(
                bt[0:1, :], bin_[:].rearrange("(p f) -> p f", p=1)
            )
            rep = 1
            while rep < 32:
                nc.gpsimd.dma_start(bt[rep : 2 * rep, :], bt[0:rep, :])
                rep *= 2
            ones_t = cp.tile([1, 128], bf16)
            nc.vector.memset(ones_t[:], 1.0)
            inv32_t = cp.tile([32, 128], bf16)
            nc.vector.memset(inv32_t[:], 1.0 / 32.0)

            x_ofs = w_ofs = o_ofs = 0
            bank_g = 0  # global bank index within core
            for k, (P, NB) in enumerate(zip(CHUNK_POS, CHUNK_BANKS)):
                xt = xp.tile([32, max_p * 128], bf16, tag="xt")
                nc.sync.dma_start(
                    xt[:, : P * 128],
                    xin[x_ofs : x_ofs + P * C * B].rearrange(
                        "(p f) -> p f", p=32
                    ),
                )
                wt = wp.tile([32, max_p * 32], bf16, tag="wt")
                nc.sync.dma_start(
                    wt[:, : P * 32],
                    win[w_ofs : w_ofs + P * C * C].rearrange(
                        "(p f) -> p f", p=32
                    ),
                )
                ot = op.tile([128, max_p * 32], bf16, tag="ot")
                seg_start = 0
                for b in range(NB):
                    gb = bank_g + b
                    pt = ps.tile([128, 512], f32)
                    # bias seeds the bank first (start=True zeroes the whole
                    # 2KB PSUM bank lazily); x matmuls then accumulate.
                    # Bank 0 reads the raw partition-0 row (no tree
                    # dependency); later banks use the 32-partition replica
                    # at full streaming rate.
                    if gb == 0:
                        nc.tensor.matmul(
                            pt[:, :],
                            ones_t[0:1, 0:128],
                            bt[0:1, 0:512],
                            start=True,
                            stop=False,
                            tile_position=(0, 0),
                            skip_group_check=True,
                        )
                    else:
                        nc.tensor.matmul(
                            pt[:, :],
                            inv32_t[0:32, 0:128],
                            bt[0:32, gb * 512 : (gb + 1) * 512],
                            start=True,
                            stop=False,
                            tile_position=(0, 0),
                            skip_group_check=True,
                        )
                    for t in range(BANK_POS):
                        p = b * BANK_POS + t
                        nc.tensor.matmul(
                            pt[:, t * 32 : (t + 1) * 32],
                            xt[0:32, p * 128 : (p + 1) * 128],
                            wt[0:32, p * 32 : (p + 1) * 32],
                            start=False,
                            stop=(t == BANK_POS - 1),
                            tile_position=(0, 0),
                            skip_group_check=True,
                        )
                    dst = ot[:, b * 512 : (b + 1) * 512]
                    if gb % 2 == 0:
                        nc.vector.tensor_copy(dst, pt[:])
                    else:
                        nc.scalar.copy(dst, pt[:])
                    if (b + 1 - seg_start >= SEG_BANKS) or b == NB - 1:
                        nseg = (b + 1 - seg_start) * 512
                        seng = nc.gpsimd
                        if k >= len(CHUNK_POS) - 3:
                            seng = nc.sync if (b // SEG_BANKS) % 2 else nc.gpsimd
                        seng.dma_start(
                            oout[o_ofs : o_ofs + 128 * nseg].rearrange(
                                "(p f) -> p f", p=128
                            ),
                            ot[:, seg_start * 512 : (b + 1) * 512],
                        )
                        o_ofs += 128 * nseg
                        seg_start = b + 1
                x_ofs += P * C * B
                w_ofs += P * C * C
                bank_g += NB
    _split_multi_waits(nc)
    return nc


def _get_nc():
    if "nc" not in _CACHE:
        _CACHE["nc"] = _build_nc()
    return _CACHE["nc"]


def _prep(x, weight, bias):
    import ml_dtypes

    bf16 = ml_dtypes.bfloat16
    x = np.ascontiguousarray(x, dtype=np.float32).reshape(B, C, L).astype(bf16)
    weight = np.asarray(weight, dtype=np.float32).reshape(L, C, C).astype(bf16)
    bias = np.asarray(bias, dtype=np.float32).reshape(L, C).astype(bf16)
    xins, wins, bins = [], [], []
    for m in range(N_CORES):
        xc, wc = [], []
        ofs = m * L_CORE
        bins.append(np.ascontiguousarray(bias[ofs : ofs + L_CORE].reshape(-1)))
        for P in CHUNK_POS:
            # x chunk: [b, c, P] -> [c, (pos, b)] flattened
            xs = x[:, :, ofs : ofs + P]
            xc.append(np.transpose(xs, (1, 2, 0)).reshape(-1))
            # w chunk: [P, c, c'] -> [c, (pos, c')]
            ws = weight[ofs : ofs + P]
            wc.append(np.transpose(ws, (1, 0, 2)).reshape(-1))
            ofs += P
        xins.append(np.concatenate(xc))
        wins.append(np.concatenate(wc))
    return np.stack(xins), np.stack(wins), np.stack(bins)


def _segments(NB):
    """Store-segment sizes (in banks) the kernel emits for an NB-bank chunk."""
    segs = []
    seg_start = 0
    for b in range(NB):
        if (b + 1 - seg_start >= SEG_BANKS) or b == NB - 1:
            segs.append(b + 1 - seg_start)
            seg_start = b + 1
    return segs


def _post(outs):
    out = np.empty((B, C, L), np.float32)
    for m in range(N_CORES):
        flat = np.asarray(outs[m], dtype=np.float32)
        fofs = 0
        lofs = m * L_CORE
        for NB in CHUNK_BANKS:
            for sb in _segments(NB):
                sp = sb * BANK_POS  # positions in this segment
                n = 128 * sb * 512
                # [b, (pos, c')] -> out[b, c', lofs + pos]
                seg = flat[fofs : fofs + n].reshape(B, sp, C)
                out[:, :, lofs : lofs + sp] = np.transpose(seg, (0, 2, 1))
                fofs += n
                lofs += sp
    return np.ascontiguousarray(out.reshape(B, C, H, W))


def _get_runner():
    """Cached shard_map executable (run_bass_via_pjrt re-jits every call;
    repeat kernel() invocations only pay transfer + execute with this)."""
    if "runner" in _CACHE:
        return _CACHE["runner"]
    import jax
    import jax.numpy as jnp  # noqa: F401
    from jax.sharding import Mesh, PartitionSpec
    from jax.experimental.shard_map import shard_map
    import concourse.mybir as mybir
    from concourse import bass2jax

    nc = _get_nc()
    bass2jax.install_neuronx_cc_hook()
    part_name = nc.partition_id_tensor.name if nc.partition_id_tensor else None
    in_names, out_names, out_avals = [], [], []
    for alloc in nc.m.functions[0].allocations:
        if not isinstance(alloc, mybir.MemoryLocationSet):
            continue
        name = alloc.memorylocations[0].name
        if alloc.kind == "ExternalInput":
            if name != part_name:
                in_names.append(name)
        elif alloc.kind == "ExternalOutput":
            out_names.append(name)
            out_avals.append(
                jax.core.ShapedArray(
                    tuple(alloc.tensor_shape), mybir.dt.np(alloc.dtype)
                )
            )
    n_params = len(in_names)
    all_names = in_names + out_names
    if part_name is not None:
        all_names = all_names + [part_name]
    all_names = tuple(all_names)

    def _body(*args):
        operands = list(args)
        if part_name is not None:
            operands.append(bass2jax.partition_id_tensor())
        return tuple(
            bass2jax._bass_exec_p.bind(
                *operands,
                out_avals=tuple(out_avals),
                in_names=all_names,
                out_names=tuple(out_names),
                lowering_input_output_aliases=(),
                sim_require_finite=True,
                sim_require_nnan=True,
                nc=nc,
            )
        )

    devices = jax.devices()[:N_CORES]
    mesh = Mesh(np.asarray(devices), ("core",))
    n_outs = len(out_names)
    sharded = jax.jit(
        shard_map(
            _body,
            mesh=mesh,
            in_specs=(PartitionSpec("core"),) * (n_params + n_outs),
            out_specs=(PartitionSpec("core"),) * n_outs,
            check_rep=False,
        ),
        donate_argnums=tuple(range(n_params, n_params + n_outs)),
        keep_unused=True,
    )

    def run(in_maps):
        concat_in = [
            np.concatenate([np.asarray(m[nm]) for m in in_maps], axis=0)
            for nm in in_names
        ]
        concat_zeros = [
            np.zeros((N_CORES * a.shape[0], *a.shape[1:]), a.dtype)
            for a in out_avals
        ]
        outs = sharded(*concat_in, *concat_zeros)
        return [
            {
                nm: np.asarray(outs[i]).reshape(N_CORES, *out_avals[i].shape)[c]
                for i, nm in enumerate(out_names)
            }
            for c in range(N_CORES)
        ]

    _CACHE["runner"] = run
    return run


def run_spmd(in_maps, trace=False):
    nc = _get_nc()
    if trace:
        from concourse.bass_utils import run_bass_kernel_spmd

        return run_bass_kernel_spmd(nc, in_maps, list(range(N_CORES)), trace=True)

    class _Res:
        pass

    res = _Res()
    res.results = _get_runner()(in_maps)
    res.exec_time_ns = None
    res.instructions_and_trace = None
    return res


def kernel(x, px, weight, bias, _trace=False, _return_meta=None):
    x = np.asarray(x, dtype=np.float32)
    weight = np.asarray(weight, dtype=np.float32)
    bias = np.asarray(bias, dtype=np.float32)
    xin, win, bin_ = _prep(x, weight, bias)
    in_maps = [
        {"xin": xin[m], "win": win[m], "bin": bin_[m]} for m in range(N_CORES)
    ]
    res = run_spmd(in_maps, trace=_trace)
    out = _post([res.results[m]["oout"] for m in range(N_CORES)])
    if _return_meta is not None:
        _return_meta["exec_time_ns"] = res.exec_time_ns
        _return_meta["trace"] = res.instructions_and_trace
    return out
